# revision 21
# baseline (speedup 1.0000x reference)
"""Multi-head attention kernel for Trainium2, SPMD across 8 NeuronCores.

Problem: b=2, s=2048, d_model=1024, 16 heads x 64 dims, packed QKV proj,
softmax over keys (boolean key mask), out-projection.

Sharding: core c in 0..7 handles batch b = c//4 and a group of 4 heads
g = c%4 (data parallel over batch x head/tensor parallel).  Each core
computes its head-group's out-projection partial [2048, 1024]; the host
sums the 4 partials per batch (the row-parallel reduction) and upcasts
from bf16.

Key scheduling facts (from NTFF traces):
  - The PE streams ~2 cols/ns (power-throttle-capped ~83% of 2.4GHz,
    with 8-16us bursts duty-cycled to 50% once the die heats up); total
    matmul column count (~390k) sets a ~165-200us floor.
  - The attention inner loop is SCALAR-bound: 2 exps of [128,1024] per
    key tile = 2.56us vs 2.08us of PE work.  Do NOT fuse both heads
    into one [128,2048] ACT: the single-buffered PSUM read serializes
    St(k+1) behind the whole 2.2us exp (1.15us PE bubble per k).  Two
    per-head tiles ping-ponged through psA (bufs=2) stagger the exp
    reads instead.
  - fp8 (any stage) fails the 2e-2 gate: measured 4-8e-2 absmax-rel.
  - DMA triggers cost ~0.7us each on the issuing queue; input loads are
    split across the two HWDGE queues (sync + scalar); ~6MB of input at
    ~358GB/s makes the first ~17us DMA-bandwidth-bound.
  - Cross-loop PREHEAT: the next loop's first st_exp's are emitted into
    the current loop's PV drain so the scalar engine never starves at
    loop boundaries.
  - The normalize chain (copy rowsum -> DMA hop to p0 -> gpsimd
    broadcast -> reciprocal -> mul -> head-B pack DMA) is ~7us; out-proj
    tiles that consume a chain's output must be scheduled well after it
    or their LDWEIGHTS head-of-line-blocks the PE FIFO.

Device-side dataflow per core (bf16 matmul operands, fp32 PSUM):
  - QKV proj, weights stationary.  Q,K produced transposed [d, s], two
    heads packed per SBUF tile (head A rows 0-63, head B rows 64-127).
    V produced in natural layout [s, d] as 16 tiles [128, 4*65] with a
    ones-column per head (col 64) that makes the PV matmul also produce
    the softmax rowsum.  Masked key rows of V (and the ones col) are
    zeroed via a per-partition scalar multiply == exact -inf masking.
  - Attention loops j-major: (p0,j0 +V inject, pipe=16, 1024-wide),
    then SIX 512-chunk loops (p1-j0 x2, p0-j1 x2, p1-j1 x2) with
    fused-head [128,1024] St tiles; chunking costs no extra scalar time
    and releases acc slots mid-loop, killing cross-loop PSUM waits.
    Out-proj tiles inject into the chunk loops' exp-pacing slack; the
    last 4 tiles' pair-0 halves prehoist to SBUF partials so the tail
    runs only pair-1 matmuls + DVE add-evicts after the final chain.
  - PV: out^T[65, sq] accumulated per (head, 512-chunk) in 1-bank PSUM
    accs; row 64 = rowsum.  PSUM budget: psA 2x[128,1024] (St / proj) +
    psB 4x 1-bank (psv / accs / po) = 8 banks exactly.
  - normalize per 512-chunk: rowsum row -> SBUF (DVE), DMA-hop to
    partition 0, gpsimd partition_broadcast, reciprocal_approx_fast,
    multiply -> O^T packed per pair (head B staged via scr + DMA into
    rows 64-127).
  - out-proj per s-row-tile: po [128,512] psum, stationary = packed O^T
    s-slices, moving = W_out^T, both pairs accumulated; evict split
    across ScalarE/DVE into [128, 4096] group buffers; one output DMA
    per 4 tiles (3D access pattern).
"""

import numpy as np
import ml_dtypes

BF = ml_dtypes.bfloat16
S = 2048
C = 1024
DQ = 64
HL = 4  # local heads per core
KT = S // 128  # 16 key tiles
CT = C // 128  # 8 contraction tiles
SCALE = 8.0  # sqrt(DQ)

_CACHED = None


def _build():
    import concourse.bacc as bacc
    import concourse.mybir as mybir
    import concourse.tile as tile

    F32 = mybir.dt.float32
    BF16 = mybir.dt.bfloat16
    EXP = mybir.ActivationFunctionType.Exp

    nc = bacc.Bacc(
        "TRN2",
        target_bir_lowering=False,
        debug=False,
        enable_asserts=False,
        num_devices=8,
    )

    XT = nc.dram_tensor("xt", [C, S], BF16, kind="ExternalInput").ap()
    WQ = nc.dram_tensor("wq", [128, CT * 256], BF16, kind="ExternalInput").ap()
    WK = nc.dram_tensor("wk", [128, CT * 256], BF16, kind="ExternalInput").ap()
    WV = nc.dram_tensor("wv", [128, CT * 256], BF16, kind="ExternalInput").ap()
    WO = nc.dram_tensor("wo", [HL * DQ, C], BF16, kind="ExternalInput").ap()
    MV = nc.dram_tensor("maskv", [128, KT], F32, kind="ExternalInput").ap()
    OUT = nc.dram_tensor("out", [S, C], BF16, kind="ExternalOutput").ap()

    with tile.TileContext(nc) as tc:
        with (
            tc.tile_pool(name="xt", bufs=CT) as p_xt,
            tc.tile_pool(name="w", bufs=3) as p_w,
            tc.tile_pool(name="wo", bufs=2) as p_wo,
            tc.tile_pool(name="cst", bufs=1) as p_c,
            tc.tile_pool(name="qk", bufs=4) as p_qk,
            tc.tile_pool(name="v", bufs=KT) as p_v,
            tc.tile_pool(name="pt", bufs=36) as p_pt,
            tc.tile_pool(name="r", bufs=4) as p_r,
            tc.tile_pool(name="bc", bufs=4) as p_bc,
            tc.tile_pool(name="ot", bufs=2) as p_ot,
            tc.tile_pool(name="sc", bufs=2) as p_sc,
            tc.tile_pool(name="os", bufs=2) as p_os,
            tc.tile_pool(name="pp", bufs=8) as p_pp,
            tc.tile_pool(name="psA", bufs=2, space="PSUM") as psA,
            tc.tile_pool(name="psB", bufs=4, space="PSUM") as psB,
        ):
            # ---------------- input DMA ----------------
            # Two HWDGE queues: sync gets the q-projection critical path
            # (wq + xt), scalar gets everything needed later (wk, wv,
            # mask, wo).  Triggers cost ~0.7us each on the issuing queue.
            wq_sb = p_w.tile([128, CT * 256], BF16, tag="w", name="wq_sb")
            wk_sb = p_w.tile([128, CT * 256], BF16, tag="w", name="wk_sb")
            wv_sb = p_w.tile([128, CT * 256], BF16, tag="w", name="wv_sb")
            xt_t = [p_xt.tile([128, S], BF16, tag="xt", name="xt_t") for _ in range(CT)]
            nc.sync.dma_start(wq_sb[:, 0:512], WQ[:, 0:512])
            nc.scalar.dma_start(xt_t[0][:, 1024:2048], XT[0:128, 1024:2048])
            nc.sync.dma_start(xt_t[0][:, 0:1024], XT[0:128, 0:1024])
            nc.scalar.dma_start(wk_sb[:, 0:512], WK[:, 0:512])
            nc.sync.dma_start(wq_sb[:, 512 : CT * 256], WQ[:, 512 : CT * 256])
            for c in range(1, CT):
                eng = nc.sync if c % 2 else nc.scalar
                eng.dma_start(xt_t[c][:], XT[c * 128 : (c + 1) * 128, :])
            nc.scalar.dma_start(wk_sb[:, 512 : CT * 256], WK[:, 512 : CT * 256])
            nc.scalar.dma_start(wv_sb[:], WV[:])
            mv_t = p_c.tile([128, KT], F32, tag="mv", name="mv_t")
            nc.scalar.dma_start(mv_t[:], MV[:])
            wo_t = []
            for p in range(2):
                t = p_wo.tile([128, C], BF16, tag="wo", name="wo_t")
                nc.scalar.dma_start(t[:], WO[p * 128 : (p + 1) * 128, :])
                wo_t.append(t)

            # ---------------- QKV projection ----------------
            # Alternate PSUM pools per call: psA holds one [128,2048]
            # (tag A, 4 banks), psB-calls use 4x [128,512] (tag B, 1 bank
            # each).  Evictions split ScalarE/DVE so the next call's
            # PSUM frees fast.
            qk_tiles = {}

            def evict(dst, src, idx):
                if idx % 2 == 0:
                    nc.vector.tensor_copy(dst, src)
                else:
                    nc.scalar.copy(dst, src)

            def proj_qk_A(nm, wsb, pair):
                dst = p_qk.tile([128, S], BF16, tag="qk", name="qk_t")
                qk_tiles[(nm, pair)] = dst
                ps = [psA.tile([128, 1024], F32, tag="A", name="pjA") for _ in range(2)]
                for c in range(CT):
                    wt = wsb[:, c * 256 + pair * 128 : c * 256 + (pair + 1) * 128]
                    for q in range(4):
                        nc.tensor.matmul(
                            ps[q // 2][:, (q % 2) * 512 : (q % 2 + 1) * 512],
                            lhsT=wt,
                            rhs=xt_t[c][:, q * 512 : (q + 1) * 512],
                            start=(c == 0),
                            stop=(c == CT - 1),
                        )
                for q in range(4):
                    evict(
                        dst[:, q * 512 : (q + 1) * 512],
                        ps[q // 2][:, (q % 2) * 512 : (q % 2 + 1) * 512],
                        q,
                    )

            def proj_qk_B(nm, wsb, pair):
                dst = p_qk.tile([128, S], BF16, tag="qk", name="qk_t")
                qk_tiles[(nm, pair)] = dst
                ps = [psB.tile([128, 512], F32, tag="B", name="pjB") for _ in range(4)]
                for c in range(CT):
                    wt = wsb[:, c * 256 + pair * 128 : c * 256 + (pair + 1) * 128]
                    for q in range(4):
                        nc.tensor.matmul(
                            ps[q][:, 0:512],
                            lhsT=wt,
                            rhs=xt_t[c][:, q * 512 : (q + 1) * 512],
                            start=(c == 0),
                            stop=(c == CT - 1),
                        )
                for q in range(4):
                    evict(dst[:, q * 512 : (q + 1) * 512], ps[q][:, 0:512], q)

            proj_qk_A("q", wq_sb, 0)
            proj_qk_B("k", wk_sb, 0)
            proj_qk_A("q", wq_sb, 1)

            # ---------------- V projection (injected into loop 0) ------
            v_t = []

            def proj_v_tile(st):
                psv = psB.tile([128, HL * DQ], F32, tag="B", name="psv")
                for c in range(CT):
                    nc.tensor.matmul(
                        psv[:, 0 : HL * DQ],
                        lhsT=xt_t[c][:, st * 128 : (st + 1) * 128],
                        rhs=wv_sb[:, c * 256 : c * 256 + HL * DQ],
                        start=(c == 0),
                        stop=(c == CT - 1),
                    )
                vt = p_v.tile([128, HL * 65], BF16, tag="v", name="v_t")
                v3 = vt[:, 0 : HL * 65].rearrange("p (h c) -> p h c", c=65)
                s3 = psv[:, 0 : HL * DQ].rearrange("p (h c) -> p h c", c=DQ)
                nc.vector.tensor_copy(v3[:, :, 0:DQ], s3[:, :, :])
                nc.vector.memset(v3[:, :, DQ : DQ + 1], 1.0)
                nc.vector.tensor_scalar_mul(vt[:], vt[:], mv_t[:, st : st + 1])
                v_t.append(vt)

            # ---------------- out-projection tile ----------------------
            ot_tiles = {}
            os_groups = {}
            OUT3 = OUT.rearrange("(t p) c -> p t c", p=128)

            def emit_st_tile(st, evict_eng="v"):
                # evict_eng: "v" = DVE both halves (in-loop; scalar busy
                # with exps), "s" = scalar both (chain window; DVE busy),
                # "vs" = split across both (tail; both engines free).
                g = st // 2
                if g not in os_groups:
                    os_groups[g] = p_os.tile([128, 2048], BF16, tag="os", name="os_g")
                osb = os_groups[g]
                lo = (st % 2) * 1024
                for n in range(2):
                    po = psB.tile([128, 512], F32, tag="B", name="po")
                    for p in range(2):
                        nc.tensor.matmul(
                            po[:, 0:512],
                            lhsT=ot_tiles[p][:, st * 128 : (st + 1) * 128],
                            rhs=wo_t[p][:, n * 512 : (n + 1) * 512],
                            start=(p == 0),
                            stop=(p == 1),
                        )
                    dst = osb[:, lo + n * 512 : lo + (n + 1) * 512]
                    if evict_eng == "s" or (evict_eng == "vs" and n == 1):
                        nc.scalar.copy(dst, po[:, 0:512])
                    else:
                        nc.vector.tensor_copy(dst, po[:, 0:512])
                os3 = osb.rearrange("p (t c) -> p t c", c=1024)
                if st >= 14:
                    # last two tiles: fire per-tile so the final output
                    # DMA isn't gated on both tiles' evicts
                    nc.sync.dma_start(
                        OUT3[:, st : st + 1, :], os3[:, st % 2 : st % 2 + 1, :]
                    )
                elif st % 2 == 1:
                    nc.sync.dma_start(OUT3[:, 2 * g : 2 * g + 2, :], os3[:, :, :])

            # ---------------- attention loops ---------------------------
            # Each loop covers W sq columns for one head-pair.  St tiles
            # are [128,1024] from psA (per-head when W=1024, fused heads
            # when W=512); exp is one ACT per tile; PV accumulates per
            # (head, 512-chunk) into 1-bank psB accs, row 64 = rowsum.
            # Loops are emitted with cross-loop PREHEAT: the next loop's
            # first st_exp's are interleaved into the current loop's PV
            # drain so the scalar engine never starves at boundaries.
            class Loop:
                def __init__(self, pair, jo, W, pipe, inject=None):
                    self.pair, self.jo, self.W, self.pipe = pair, jo, W, pipe
                    self.inject = inject
                    self.qt = qk_tiles[("q", pair)]
                    self.kt = qk_tiles[("k", pair)]
                    self.ot = ot_tiles[pair]
                    self.scr = p_sc.tile([64, 1024], BF16, tag="sc", name="sc_t")
                    self.nch = W // 512
                    self.accs = {}
                    self.pts = {}

                def st_exp(self, k):
                    W, jo, kt, qt = self.W, self.jo, self.kt, self.qt
                    if W == 1024:
                        for i, base in enumerate((0, 64)):
                            stb = psA.tile([128, 1024], F32, tag="A", name="stb")
                            for n in range(2):
                                nc.tensor.matmul(
                                    stb[:, n * 512 : (n + 1) * 512],
                                    lhsT=kt[base : base + DQ, k * 128 : (k + 1) * 128],
                                    rhs=qt[
                                        base : base + DQ,
                                        jo + n * 512 : jo + (n + 1) * 512,
                                    ],
                                    start=True,
                                    stop=True,
                                )
                            pt = p_pt.tile([128, 1024], BF16, tag="pt", name="pt_t")
                            nc.scalar.activation(pt[:], stb[:], EXP, scale=1.0 / SCALE)
                            self.pts[(k, i)] = pt
                    else:
                        stb = psA.tile([128, 1024], F32, tag="A", name="stb")
                        for i, base in enumerate((0, 64)):
                            nc.tensor.matmul(
                                stb[:, i * 512 : (i + 1) * 512],
                                lhsT=kt[base : base + DQ, k * 128 : (k + 1) * 128],
                                rhs=qt[base : base + DQ, jo : jo + 512],
                                start=True,
                                stop=True,
                            )
                        pt = p_pt.tile([128, 1024], BF16, tag="pt", name="pt_t")
                        nc.scalar.activation(pt[:], stb[:], EXP, scale=1.0 / SCALE)
                        self.pts[(k, 0)] = self.pts[(k, 1)] = pt

                def pv(self, k):
                    if not self.accs:
                        for i in range(2):
                            for n in range(self.nch):
                                self.accs[(i, n)] = psB.tile(
                                    [65, 512], F32, tag="B", name="acc"
                                )
                    for i, h in enumerate((2 * self.pair, 2 * self.pair + 1)):
                        pt = self.pts[(k, i)]
                        for n in range(self.nch):
                            rhs = (
                                pt[:, n * 512 : (n + 1) * 512]
                                if self.W == 1024
                                else pt[:, i * 512 : (i + 1) * 512]
                            )
                            nc.tensor.matmul(
                                self.accs[(i, n)][0:65, 0:512],
                                lhsT=v_t[k][:, h * 65 : h * 65 + 65],
                                rhs=rhs,
                                start=(k == 0),
                                stop=(k == KT - 1),
                            )
                    del self.pts[(k, 0)], self.pts[(k, 1)]

                def normalize(self, split=1):
                    # split=2 emits 256-col chainlets so consumers of the
                    # LAST chunk (tail out-proj tiles) unblock earlier.
                    w = 512 // split
                    for n in range(self.nch):
                        for sub in range(split):
                            cols = self.jo + n * 512 + sub * w
                            lo = sub * w
                            for i in range(2):
                                acc = self.accs[(i, n)]
                                rth = p_r.tile([65, 512], F32, tag="r", name="r_t")
                                bct = p_bc.tile([64, 512], F32, tag="bc", name="bc_t")
                                bc2 = p_bc.tile([64, 512], F32, tag="bc", name="bc2_t")
                                nc.vector.tensor_copy(
                                    rth[64:65, 0:w], acc[64:65, lo : lo + w]
                                )
                                nc.sync.dma_start(rth[0:1, 0:w], rth[64:65, 0:w])
                                nc.gpsimd.partition_broadcast(
                                    bct[0:64, 0:w], rth[0:1, 0:w]
                                )
                                nc.vector.reciprocal_approx_fast(
                                    bc2[0:64, 0:w], bct[0:64, 0:w]
                                )
                                dst = (
                                    self.ot[0:64, cols : cols + w]
                                    if i == 0
                                    else self.scr[0:64, n * 512 + lo : n * 512 + lo + w]
                                )
                                nc.vector.tensor_mul(
                                    dst, acc[0:64, lo : lo + w], bc2[0:64, 0:w]
                                )
                            nc.sync.dma_start(
                                self.ot[64:128, cols : cols + w],
                                self.scr[0:64, n * 512 + lo : n * 512 + lo + w],
                            )
                    self.accs.clear()

            for pair in range(2):
                ot_tiles[pair] = p_ot.tile([128, S], BF16, tag="ot", name="ot_t")

            # pair-0 halves of the last 4 out-proj tiles, prehoisted into
            # earlier loops' slack; the tail then only needs the pair-1
            # matmul plus an add-evict (DVE) after the final chain.
            po_part = {}

            def emit_st_p0(st):
                for n in range(2):
                    po = psB.tile([128, 512], F32, tag="B", name="pop")
                    nc.tensor.matmul(
                        po[:, 0:512],
                        lhsT=ot_tiles[0][:, st * 128 : (st + 1) * 128],
                        rhs=wo_t[0][:, n * 512 : (n + 1) * 512],
                        start=True,
                        stop=True,
                    )
                    sb = p_pp.tile([128, 512], BF16, tag="pp", name="pp_t")
                    nc.vector.tensor_copy(sb[:, 0:512], po[:, 0:512])
                    po_part[(st, n)] = sb

            def emit_st_p1(st):
                g = st // 2
                if g not in os_groups:
                    os_groups[g] = p_os.tile([128, 2048], BF16, tag="os", name="os_g")
                osb = os_groups[g]
                lo = (st % 2) * 1024
                for n in range(2):
                    po = psB.tile([128, 512], F32, tag="B", name="po")
                    nc.tensor.matmul(
                        po[:, 0:512],
                        lhsT=ot_tiles[1][:, st * 128 : (st + 1) * 128],
                        rhs=wo_t[1][:, n * 512 : (n + 1) * 512],
                        start=True,
                        stop=True,
                    )
                    dst = osb[:, lo + n * 512 : lo + (n + 1) * 512]
                    nc.vector.tensor_add(dst, po[:, 0:512], po_part[(st, n)][:, 0:512])
                os3 = osb.rearrange("p (t c) -> p t c", c=1024)
                if st >= 14:
                    nc.sync.dma_start(
                        OUT3[:, st : st + 1, :], os3[:, st % 2 : st % 2 + 1, :]
                    )
                elif st % 2 == 1:
                    nc.sync.dma_start(OUT3[:, 2 * g : 2 * g + 2, :], os3[:, :, :])

            def make_inj(table):
                def inj(k):
                    if k in table:
                        v = table[k]
                        if isinstance(v, tuple):
                            emit_st_p0(v[1])
                        else:
                            emit_st_tile(v)
                return inj

            def inj_v(k):
                # 16 V tiles over main-loop ks 2..15 (L0's first two k's
                # are preheated between the q1/k1 projection calls so the
                # exp stream starts ~8us earlier)
                proj_v_tile(len(v_t))
                if k >= 14:
                    proj_v_tile(len(v_t))

            L0 = Loop(0, 0, 1024, 16, inject=inj_v)
            L0.st_exp(0)
            L0.st_exp(1)
            proj_qk_B("k", wk_sb, 1)

            # All post-L0 loops are 512-chunk loops: identical scalar cost
            # to 1024-wide (same exp elems/instr), shorter normalize
            # chains, and acc slots release mid-loop so the next loop's
            # PV/out-proj never wait a full chain.
            loops = [
                L0,
                Loop(1, 0, 512, 12),
                Loop(1, 512, 512, 6),
                Loop(0, 1024, 512, 6, inject=make_inj({9: 0, 11: 1, 13: 2})),
                Loop(0, 1536, 512, 6, inject=make_inj({3: 3, 6: 4, 9: 5})),
                Loop(1, 1024, 512, 6, inject=make_inj({6: 6, 10: 7, 12: ("p0", 12), 14: ("p0", 13)})),
                Loop(1, 1536, 512, 4, inject=make_inj({5: ("p0", 14), 7: ("p0", 15), 11: 10, 13: 11})),
            ]
            # next-loop st_exp's emitted inside this loop's drain
            preheat = [12, 5, 3, 3, 3, 3, 0]

            for li, L in enumerate(loops):
                pre = preheat[li - 1] if li else 2
                for k in range(pre, KT):
                    L.st_exp(k)
                    if L.inject is not None:
                        L.inject(k)
                    if k >= L.pipe:
                        L.pv(k - L.pipe)
                # drain, interleaving the next loop's first st_exp's
                nxt = loops[li + 1] if li + 1 < len(loops) else None
                npre = preheat[li]
                drain = list(range(max(0, KT - L.pipe), KT))
                for di, k in enumerate(drain):
                    L.pv(k)
                    # spread the preheats over the back half of the drain
                    if nxt is not None and di >= len(drain) - npre:
                        nxt.st_exp(di - (len(drain) - npre))
                if nxt is not None:
                    L.normalize()  # last loop's normalize is emitted below

            # st6/7 deferred to run on the PE while L3b's normalize chain
            # (DVE/gpsimd/DMA) executes; scalar evicts keep the DVE free
            # for the chain.  Then the last 4 tiles.
            for st in (8, 9):
                emit_st_tile(st, evict_eng="s")
            loops[6].normalize(split=2)
            for st in range(12, 16):
                emit_st_p1(st)
    nc.compile()
    return nc


def _get_nc():
    global _CACHED
    if _CACHED is None:
        _CACHED = _build()
    return _CACHED


def _prep_in_maps(X, W_qkv, W_out, mask):
    X = np.asarray(X, dtype=np.float32)
    Wqkv = np.asarray(W_qkv, dtype=np.float32)
    Wo = np.asarray(W_out, dtype=np.float32)
    m = np.asarray(mask)
    W3 = Wqkv.reshape(16, DQ, 3, C)
    in_maps = []
    for core in range(8):
        b = core // 4
        g = core % 4
        hs = slice(4 * g, 4 * g + 4)
        # pre-arrange for contiguous SBUF prestage: [128, c*256 + j]
        def prearrange(w):  # w: [HL*DQ, C] -> [128, CT*256]
            wt = w.T.astype(BF)  # [C, HL*DQ]
            return np.ascontiguousarray(
                wt.reshape(CT, 128, HL * DQ).transpose(1, 0, 2).reshape(128, CT * 256)
            )

        wq = prearrange(W3[hs, :, 0, :].reshape(HL * DQ, C))
        wk = prearrange(W3[hs, :, 1, :].reshape(HL * DQ, C))
        wv = prearrange(W3[hs, :, 2, :].reshape(HL * DQ, C))
        wo = np.ascontiguousarray(Wo[:, 256 * g : 256 * (g + 1)].T.astype(BF))
        xt = np.ascontiguousarray(X[b].T.astype(BF))
        mv = np.ascontiguousarray(m[b].astype(np.float32).reshape(KT, 128).T)
        in_maps.append(
            {"xt": xt, "wq": wq, "wk": wk, "wv": wv, "wo": wo, "maskv": mv}
        )
    return in_maps


def _run(in_maps, trace=False, **kw):
    from concourse import bass_utils

    nc = _get_nc()
    return bass_utils.run_bass_kernel_spmd(
        nc, in_maps, core_ids=list(range(8)), trace=trace, **kw
    )


def _gather(results):
    out = np.empty((2, S, C), dtype=np.float32)
    p = [r["out"].astype(np.float32) for r in results]
    out[0] = p[0] + p[1] + p[2] + p[3]
    out[1] = p[4] + p[5] + p[6] + p[7]
    return out


def kernel(X, W_qkv, W_out, mask):
    in_maps = _prep_in_maps(X, W_qkv, W_out, mask)
    res = _run(in_maps)
    return _gather(res.results)
